# revision 3
# baseline (speedup 1.0000x reference)
"""Trainium2 Bass kernel for a 3-layer tanh RNN (batch_first).

Math (per layer l):  h_t = tanh(W_ih x_t + b_ih + W_hh h_{t-1} + b_hh)
Returns (hidden [L,B,H], out [B*S, H]) like the reference.

Strategy: data-parallel over batch (B=32 -> 4 sequences per core, 8 cores).
Everything on-chip is kept transposed (H on partitions) so the sequential
scan needs no per-step transposes:
  - input projections  pre^T = W_ih^T.T @ in^T  as big GEMMs
  - scan step: psum[m] = sum_k W_hh^T[k,m].T @ h^T[k] ; h' = tanh(psum + pre_t)
Host-side numpy does all layout transforms (transposes, sharding, bf16 cast).
"""

import sys

sys.path.insert(0, "/opt/trn_rl_repo")

import numpy as np
import ml_dtypes

import concourse.bacc as bacc
import concourse.tile as tile
import concourse.mybir as mybir
from concourse.bass import ds
from concourse import bass_utils

BF16 = ml_dtypes.bfloat16

B, S, I, H, L = 32, 1024, 128, 512, 3
NC = 8          # cores
BL = B // NC    # 4 sequences per core
CH = H // 128   # 4 partition chunks of H
U = 8           # scan unroll inside the hardware loop

_CACHE = {}


def _build(s_len=S, unroll=U):
    f32 = mybir.dt.float32
    dt = mybir.dt.bfloat16
    nc = bacc.Bacc("TRN2", target_bir_lowering=False, debug=False)

    xT = nc.dram_tensor("xT", [128, BL, s_len], dt, kind="ExternalInput").ap()
    w0T = nc.dram_tensor("w0T", [128, H], dt, kind="ExternalInput").ap()
    wiT = nc.dram_tensor("wiT", [L - 1, 128, CH, H], dt, kind="ExternalInput").ap()
    whT = nc.dram_tensor("whT", [L, 128, CH, H], dt, kind="ExternalInput").ap()
    bias = nc.dram_tensor("bias", [128, L, CH], f32, kind="ExternalInput").ap()
    h0T = nc.dram_tensor("h0T", [L, 128, CH, BL, 1], dt, kind="ExternalInput").ap()

    out2T = nc.dram_tensor("out2T", [128, CH, BL, s_len], dt, kind="ExternalOutput").ap()
    hidT = nc.dram_tensor("hidT", [L, 128, CH, BL, 1], dt, kind="ExternalOutput").ap()

    TANH = mybir.ActivationFunctionType.Tanh
    PE = mybir.EngineType.PE
    ACT_E = mybir.EngineType.Activation
    DVE_E = mybir.EngineType.DVE

    from contextlib import ExitStack
    with tile.TileContext(nc) as tc, ExitStack() as ctx:
        consts = ctx.enter_context(tc.tile_pool(name="consts", bufs=1))
        psum_pool = ctx.enter_context(tc.tile_pool(name="psum", bufs=2, space="PSUM"))
        proj_psum = ctx.enter_context(tc.tile_pool(name="proj_psum", bufs=4, space="PSUM"))
        tmp_pool = ctx.enter_context(tc.tile_pool(name="tmp", bufs=2))

        xt_sb = consts.tile([128, BL, s_len], dt, tag="xt")
        nc.sync.dma_start(out=xt_sb, in_=xT)
        w0_sb = consts.tile([128, H], dt, tag="w0")
        nc.sync.dma_start(out=w0_sb, in_=w0T)
        wi_sb = []
        for j in range(L - 1):
            t_ = consts.tile([128, CH, H], dt, tag=f"wi{j}")
            nc.sync.dma_start(out=t_, in_=wiT[j])
            wi_sb.append(t_)
        wh_sb = []
        for l in range(L):
            t_ = consts.tile([128, CH, H], dt, tag=f"wh{l}")
            nc.sync.dma_start(out=t_, in_=whT[l])
            wh_sb.append(t_)
        bias_sb = consts.tile([128, L, CH], f32, tag="bias")
        nc.sync.dma_start(out=bias_sb, in_=bias)

        out_sb = [consts.tile([128, CH, BL, s_len], dt, tag=f"out{l}", name=f"out{l}") for l in range(L)]
        pre_sb = consts.tile([128, CH, BL, s_len], dt, tag="pre")
        stage = consts.tile([128, CH, BL, unroll], dt, tag="stage")

        NT = s_len // 512 if s_len >= 512 else 1
        TW = min(512, s_len)  # proj tile width along t

        for l in range(L):
            kch = 1 if l == 0 else CH
            # ---- input projection: pre[:, m, b, :] = W_ih^T.T @ in^T + bias ----
            for m in range(CH):
                for b in range(BL):
                    for nt in range(NT):
                        ps = proj_psum.tile([128, TW], f32, tag="projps")
                        for k in range(kch):
                            if l == 0:
                                lhsT = w0_sb[:, m * 128:(m + 1) * 128]
                                rhs = xt_sb[:, b, nt * TW:(nt + 1) * TW]
                            else:
                                lhsT = wi_sb[l - 1][:, k, m * 128:(m + 1) * 128]
                                rhs = out_sb[l - 1][:, k, b, nt * TW:(nt + 1) * TW]
                            nc.tensor.matmul(ps, lhsT, rhs,
                                             start=(k == 0), stop=(k == kch - 1))
                        nc.vector.tensor_scalar_add(
                            out=pre_sb[:, m, b, nt * TW:(nt + 1) * TW],
                            in0=ps,
                            scalar1=bias_sb[:, l, m:m + 1],
                        )

            # ---- sequential scan ----
            nc.sync.dma_start(out=stage[:, :, :, unroll - 1:unroll], in_=h0T[l])
            with tc.For_i(0, s_len, unroll,
                          hint_engines=(PE, DVE_E, ACT_E)) as iv:
                for u in range(unroll):
                    prev_u = unroll - 1 if u == 0 else u - 1
                    ps = psum_pool.tile([128, CH, BL, 1], f32, tag="scanps")
                    for m in range(CH):
                        for k in range(CH):
                            nc.tensor.matmul(
                                ps[:, m, :, 0],
                                wh_sb[l][:, k, m * 128:(m + 1) * 128],
                                stage[:, k, :, prev_u],
                                start=(k == 0), stop=(k == CH - 1),
                            )
                    tmp = tmp_pool.tile([128, CH, BL, 1], f32, tag="scantmp")
                    for m in range(CH):
                        nc.vector.tensor_add(
                            out=tmp[:, m, :, :],
                            in0=ps[:, m, :, :],
                            in1=pre_sb[:, m, :, ds(iv + u, 1)],
                        )
                        nc.scalar.activation(
                            out=stage[:, m, :, u],
                            in_=tmp[:, m, :, 0],
                            func=TANH,
                        )
                # flush the unrolled window into the layer output buffer
                nc.vector.tensor_copy(
                    out=out_sb[l][:, :, :, ds(iv, unroll)],
                    in_=stage,
                )

        nc.sync.dma_start(out=out2T, in_=out_sb[L - 1])
        for l in range(L):
            nc.sync.dma_start(out=hidT[l],
                              in_=out_sb[l][:, :, :, s_len - 1:s_len])

    nc.compile()
    return nc


def _prep_inputs(x, h0, w_ih0, w_ihs, w_hhs, b_ihs, b_hhs, s_len=S):
    """Build per-core in_maps (all layout transforms host-side)."""
    w0T = np.ascontiguousarray(w_ih0.T.astype(BF16))                    # [128, H]
    wiT = np.stack([
        np.ascontiguousarray(w.T.reshape(CH, 128, H).transpose(1, 0, 2))
        for w in w_ihs.astype(BF16)
    ])                                                                   # [L-1,128,CH,H]
    whT = np.stack([
        np.ascontiguousarray(w.T.reshape(CH, 128, H).transpose(1, 0, 2))
        for w in w_hhs.astype(BF16)
    ])                                                                   # [L,128,CH,H]
    bias = np.ascontiguousarray(
        (b_ihs + b_hhs).astype(np.float32).reshape(L, CH, 128).transpose(2, 0, 1)
    )                                                                    # [128,L,CH]
    in_maps = []
    for c in range(NC):
        xs = x[c * BL:(c + 1) * BL, :s_len, :]                           # [BL,s,I]
        xT = np.ascontiguousarray(xs.transpose(2, 0, 1).astype(BF16))    # [128,BL,s]
        h0s = h0[:, c * BL:(c + 1) * BL, :]                              # [L,BL,H]
        h0T = np.ascontiguousarray(
            h0s.transpose(0, 2, 1).reshape(L, CH, 128, BL, 1).transpose(0, 2, 1, 3, 4)
        ).astype(BF16)                                                   # [L,128,CH,BL,1]
        in_maps.append({
            "xT": xT, "w0T": w0T, "wiT": wiT, "whT": whT,
            "bias": bias, "h0T": h0T,
        })
    return in_maps


def _assemble(results, s_len=S):
    hidden = np.empty((L, B, H), dtype=np.float32)
    out = np.empty((B, s_len, H), dtype=np.float32)
    for c in range(NC):
        o = np.asarray(results[c]["out2T"]).astype(np.float32)           # [128,CH,BL,s]
        out[c * BL:(c + 1) * BL] = o.transpose(2, 3, 1, 0).reshape(BL, s_len, H)
        hd = np.asarray(results[c]["hidT"]).astype(np.float32)           # [L,128,CH,BL,1]
        hidden[:, c * BL:(c + 1) * BL, :] = hd[..., 0].transpose(0, 3, 2, 1).reshape(L, BL, H)
    return hidden, out.reshape(-1, H)


def kernel(x, h0, w_ih0, w_ihs, w_hhs, b_ihs, b_hhs):
    if "nc" not in _CACHE:
        _CACHE["nc"] = _build()
    nc = _CACHE["nc"]
    in_maps = _prep_inputs(x, h0, w_ih0, w_ihs, w_hhs, b_ihs, b_hhs)
    res = bass_utils.run_bass_kernel_spmd(nc, in_maps, core_ids=list(range(NC)))
    return _assemble(res.results)


# revision 14
# speedup vs baseline: 14.5838x; 14.5838x over previous
"""Trainium2 Bass kernel for a 3-layer tanh RNN (batch_first).

Math (per layer l):  h_t = tanh(W_ih x_t + b_ih + W_hh h_{t-1} + b_hh)
Returns (hidden [L,B,H], out [B*S, H]) like the reference.

Strategy: data-parallel over batch (B=32 -> 4 sequences per core, 8 cores).
Everything on-chip is kept transposed (H on partitions) so the sequential
scan needs no per-step transposes:
  - input projections  pre^T = W_ih^T.T @ in^T  as big GEMMs
  - scan step: psum[m] = sum_k W_hh^T[k,m].T @ h^T[k] ; h' = tanh(psum + pre_t)
Host-side numpy does all layout transforms (transposes, sharding, bf16 cast).
"""

import sys

sys.path.insert(0, "/opt/trn_rl_repo")

import numpy as np
import ml_dtypes

import concourse.bacc as bacc
import concourse.tile as tile
import concourse.mybir as mybir
from concourse.bass import ds
from concourse import bass_utils

BF16 = ml_dtypes.bfloat16

B, S, I, H, L = 32, 1024, 128, 512, 3
NC = 8          # cores
BL = B // NC    # 4 sequences per core
CH = H // 128   # 4 partition chunks of H
U = 16          # scan unroll inside the hardware loop

_CACHE = {}


def _build(s_len=S, unroll=U, reps=1):
    f32 = mybir.dt.float32
    dt = mybir.dt.bfloat16
    nc = bacc.Bacc("TRN2", target_bir_lowering=False, debug=False)

    xT = nc.dram_tensor("xT", [128, BL, s_len], dt, kind="ExternalInput").ap()
    w0T = nc.dram_tensor("w0T", [128, H], dt, kind="ExternalInput").ap()
    wiT = nc.dram_tensor("wiT", [L - 1, 128, CH, H], dt, kind="ExternalInput").ap()
    whT = nc.dram_tensor("whT", [L, 128, CH, H], dt, kind="ExternalInput").ap()
    bias = nc.dram_tensor("bias", [128, L, CH], f32, kind="ExternalInput").ap()
    h0T = nc.dram_tensor("h0T", [L, 128, CH, BL, 1], dt, kind="ExternalInput").ap()
    ident = nc.dram_tensor("ident", [128, 128], dt, kind="ExternalInput").ap()

    out2T = nc.dram_tensor("out2T", [128, CH, BL, s_len], dt, kind="ExternalOutput").ap()
    hidT = nc.dram_tensor("hidT", [L, 128, CH, BL, 1], dt, kind="ExternalOutput").ap()

    TANH = mybir.ActivationFunctionType.Tanh
    PE = mybir.EngineType.PE
    ACT_E = mybir.EngineType.Activation
    DVE_E = mybir.EngineType.DVE

    from contextlib import ExitStack
    with tile.TileContext(nc) as tc, ExitStack() as ctx:
        consts = ctx.enter_context(tc.tile_pool(name="consts", bufs=1))
        psum_pool = ctx.enter_context(tc.tile_pool(name="psum", bufs=2, space="PSUM"))
        proj_psum = ctx.enter_context(tc.tile_pool(name="proj_psum", bufs=4, space="PSUM"))
        tmp_pool = ctx.enter_context(tc.tile_pool(name="tmp", bufs=2))

        xt_sb = consts.tile([128, BL, s_len], dt, tag="xt")
        nc.sync.dma_start(out=xt_sb, in_=xT)
        w0_sb = consts.tile([128, H], dt, tag="w0")
        nc.sync.dma_start(out=w0_sb, in_=w0T)
        wi_sb = []
        for j in range(L - 1):
            t_ = consts.tile([128, CH, H], dt, tag=f"wi{j}")
            nc.sync.dma_start(out=t_, in_=wiT[j])
            wi_sb.append(t_)
        wh_sb = []
        for l in range(L):
            t_ = consts.tile([128, CH, H], dt, tag=f"wh{l}")
            nc.sync.dma_start(out=t_, in_=whT[l])
            wh_sb.append(t_)
        bias_sb = consts.tile([128, L, CH], f32, tag="bias")
        nc.sync.dma_start(out=bias_sb, in_=bias)
        ident_sb = consts.tile([128, 128], dt, tag="ident")
        nc.sync.dma_start(out=ident_sb, in_=ident)

        out_sb = [consts.tile([128, CH, BL, s_len], dt, tag=f"out{l}", name=f"out{l}") for l in range(L)]
        pre_sb = consts.tile([128, CH, BL, s_len], dt, tag="pre")
        stage = consts.tile([128, CH, BL, unroll], dt, tag="stage")

        NT = s_len // 512 if s_len >= 512 else 1
        TW = min(512, s_len)  # proj tile width along t

        for _rep in range(reps):
         for l in range(L):
            kch = 1 if l == 0 else CH
            # ---- input projection: pre[:, m, b, :] = W_ih^T.T @ in^T + bias ----
            for m in range(CH):
                for b in range(BL):
                    for nt in range(NT):
                        ps = proj_psum.tile([128, TW], f32, tag="projps")
                        for k in range(kch):
                            if l == 0:
                                lhsT = w0_sb[:, m * 128:(m + 1) * 128]
                                rhs = xt_sb[:, b, nt * TW:(nt + 1) * TW]
                            else:
                                lhsT = wi_sb[l - 1][:, k, m * 128:(m + 1) * 128]
                                rhs = out_sb[l - 1][:, k, b, nt * TW:(nt + 1) * TW]
                            nc.tensor.matmul(ps, lhsT, rhs,
                                             start=(k == 0), stop=(k == kch - 1))
                        nc.vector.tensor_scalar_add(
                            out=pre_sb[:, m, b, nt * TW:(nt + 1) * TW],
                            in0=ps,
                            scalar1=bias_sb[:, l, m:m + 1],
                        )

            # ---- sequential scan ----
            nc.sync.dma_start(out=stage[:, :, :, unroll - 1:unroll], in_=h0T[l])
            with tc.For_i(0, s_len, unroll,
                          hint_engines=(PE, DVE_E, ACT_E)) as iv:
                # stage the pre window once per body so all PE APs stay static
                pre_stage = tmp_pool.tile([128, CH, BL, unroll], dt, tag="prestage")
                nc.vector.tensor_copy(
                    out=pre_stage, in_=pre_sb[:, :, :, ds(iv, unroll)])
                for u in range(unroll):
                    prev_u = unroll - 1 if u == 0 else u - 1
                    ps = psum_pool.tile([128, CH, BL, 1], f32, tag="scanps")
                    for m in range(CH):
                        # seed psum with pre_t (identity matmul), then W_hh
                        nc.tensor.matmul(
                            ps[:, m, :, 0], ident_sb, pre_stage[:, m, :, u],
                            start=True, stop=False,
                        )
                        for k in range(CH):
                            nc.tensor.matmul(
                                ps[:, m, :, 0],
                                wh_sb[l][:, k, m * 128:(m + 1) * 128],
                                stage[:, k, :, prev_u],
                                start=False, stop=(k == CH - 1),
                            )
                    for m in range(CH):
                        nc.scalar.activation(
                            out=stage[:, m, :, u],
                            in_=ps[:, m, :, 0],
                            func=TANH,
                        )
                # flush the unrolled window into the layer output buffer
                nc.vector.tensor_copy(
                    out=out_sb[l][:, :, :, ds(iv, unroll)],
                    in_=stage,
                )

        nc.sync.dma_start(out=out2T, in_=out_sb[L - 1])
        for l in range(L):
            nc.sync.dma_start(out=hidT[l],
                              in_=out_sb[l][:, :, :, s_len - 1:s_len])

    nc.compile()
    return nc


def _prep_inputs(x, h0, w_ih0, w_ihs, w_hhs, b_ihs, b_hhs, s_len=S):
    """Build per-core in_maps (all layout transforms host-side)."""
    w0T = np.ascontiguousarray(w_ih0.T.astype(BF16))                    # [128, H]
    wiT = np.stack([
        np.ascontiguousarray(w.T.reshape(CH, 128, H).transpose(1, 0, 2))
        for w in w_ihs.astype(BF16)
    ])                                                                   # [L-1,128,CH,H]
    whT = np.stack([
        np.ascontiguousarray(w.T.reshape(CH, 128, H).transpose(1, 0, 2))
        for w in w_hhs.astype(BF16)
    ])                                                                   # [L,128,CH,H]
    bias = np.ascontiguousarray(
        (b_ihs + b_hhs).astype(np.float32).reshape(L, CH, 128).transpose(2, 0, 1)
    )                                                                    # [128,L,CH]
    in_maps = []
    for c in range(NC):
        xs = x[c * BL:(c + 1) * BL, :s_len, :]                           # [BL,s,I]
        xT = np.ascontiguousarray(xs.transpose(2, 0, 1).astype(BF16))    # [128,BL,s]
        h0s = h0[:, c * BL:(c + 1) * BL, :]                              # [L,BL,H]
        h0T = np.ascontiguousarray(
            h0s.transpose(0, 2, 1).reshape(L, CH, 128, BL, 1).transpose(0, 2, 1, 3, 4)
        ).astype(BF16)                                                   # [L,128,CH,BL,1]
        in_maps.append({
            "xT": xT, "w0T": w0T, "wiT": wiT, "whT": whT,
            "bias": bias, "h0T": h0T,
            "ident": np.eye(128, dtype=BF16),
        })
    return in_maps


def _assemble(results, s_len=S):
    hidden = np.empty((L, B, H), dtype=np.float32)
    out = np.empty((B, s_len, H), dtype=np.float32)
    for c in range(NC):
        o = np.asarray(results[c]["out2T"]).astype(np.float32)           # [128,CH,BL,s]
        out[c * BL:(c + 1) * BL] = o.transpose(2, 3, 1, 0).reshape(BL, s_len, H)
        hd = np.asarray(results[c]["hidT"]).astype(np.float32)           # [L,128,CH,BL,1]
        hidden[:, c * BL:(c + 1) * BL, :] = hd[..., 0].transpose(0, 3, 2, 1).reshape(L, BL, H)
    return hidden, out.reshape(-1, H)


def kernel(x, h0, w_ih0, w_ihs, w_hhs, b_ihs, b_hhs):
    if "nc" not in _CACHE:
        _CACHE["nc"] = _build()
    nc = _CACHE["nc"]
    in_maps = _prep_inputs(x, h0, w_ih0, w_ihs, w_hhs, b_ihs, b_hhs)
    res = bass_utils.run_bass_kernel_spmd(nc, in_maps, core_ids=list(range(NC)))
    return _assemble(res.results)
